# revision 31
# baseline (speedup 1.0000x reference)
"""FlowNet-style correlation layer (B=4, C=128, H=W=192, k=9, stride=1) on 8 trn2 cores.

Design (per core; cores = 4 batches x 2 H-halves, SPMD):
  - Host pre-blocks x into per-patch-contiguous layout [c, blk, 128] (bf16,
    prescaled by 1/C — exact exponent shift) and pads y to h-major
    [c, 104, 200] bf16 (no duplication).
  - Residents stream into SBUF just-in-time: x in 6 two-row chunks, y in
    7 16-row chunks, issued at the top of the row loop ahead of need, all
    on the sync DGE ring (the scalar ring stays clear for ACT evacs).
  - For each 8x16 pixel patch (144 blocks): one PE matmul contracting
    channels: lhsT = x-patch [c, 128], rhs = strided window into y
    [c, h':16, w':24] -> psum[128, 384] ("banded all-pairs":
    psum[m, n] = sum_c x[c,pix_m] * y[c,ctx_n], n = h'*24 + w').
    The per-block cadence (~390ns) sits just above the DMA pace so the
    tensor queue stays continuously busy and the PE p-state holds at
    full clock (bursty schedules demote it to 1.2 GHz).
  - Evacuate psum -> sbuf bf16 (alternating ACT/DVE), dst INTERLEAVED
    across block pairs (stride 2 elements, ~1.15x contiguous cost):
    band flat = row*4608 + pair*768 + ctx*2 + (bw%2).
  - Slab write: partition half hl<4 only ever needs ctx rows 0..11 and
    hl>=4 needs rows 4..15 (288 of 384 ctx cols); interleaving makes each
    pair's slab contiguous (1152B runs) and two-row band tiles make the
    DRAM side fully contiguous (6912B descriptors): per TWO block rows
    one upper + one lower slab DMA -> 10.6 MB written (3.56x of the
    2.99 MB useful) instead of the 14.2 MB full band (4.74x).
  - Host gathers each pixel's 81 useful context columns
    (col = (bw//2)*576 + ((hl+di-4*(hl>=4))*24 + wl+dj)*2 + bw%2,
    pure indexing, bit-identical) and reassembles [B, 81, 192, 192] f32.
"""

import numpy as np

B, C, H, W = 4, 128, 192, 192
K = 9                      # kernel_size
PAD = 4                    # displacement radius
NCORES = 8
HSH = H // 2               # 96 rows per core
YH, YW = HSH + 2 * PAD, W + 2 * PAD       # 104, 200
PH, PW = 8, 16             # patch shape (128 pixels)
CH, CW = PH + 2 * PAD, PW + 2 * PAD       # context 16 x 24
NCTX = CH * CW             # 384 band columns
NBH, NBW = HSH // PH, W // PW             # 12 x 12 = 144 blocks
NBLK = NBH * NBW
K2 = K * K                 # 81
G = 2                      # blocks per interleave group
NG = NBW // G              # groups per block row
SLABC = 12 * CW            # 288 ctx cols per partition-half slab
ROWW = NBW * SLABC         # 3456 output elements per partition per block row
NB2 = NBH // 2             # 6 two-row bands
NYC = 7                    # y chunks: 6x16 rows + 1x8

_nc_cache = None


def _build_nc():
    import concourse.bacc as bacc
    import concourse.mybir as mybir
    import concourse.tile as tile

    bf16 = mybir.dt.bfloat16
    f32 = mybir.dt.float32

    nc = bacc.Bacc("TRN2", target_bir_lowering=False, debug=False)
    x_d = nc.dram_tensor("x", [C, NBLK * 128], bf16, kind="ExternalInput")
    y_d = nc.dram_tensor("y", [C, YH * YW], bf16, kind="ExternalInput")
    out_d = nc.dram_tensor("out", [NBH, 128, ROWW], bf16,
                           kind="ExternalOutput")

    with tile.TileContext(nc) as tc:
        with (
            tc.tile_pool(name="xres", bufs=1) as x_pool,
            tc.tile_pool(name="yres", bufs=1) as y_pool,
            tc.tile_pool(name="psum", bufs=8, space="PSUM") as psum_pool,
            tc.tile_pool(name="band", bufs=6) as band_pool,
        ):
            y_sb = y_pool.tile([C, YH * YW], bf16)
            y3 = y_sb[:].rearrange("c (h w) -> c h w", w=YW)
            XROW = NBW * 128                               # 1536
            xt = [x_pool.tile([C, 2 * XROW], bf16, name=f"xt{r}")
                  for r in range(NB2)]

            def load_x(b):
                nc.sync.dma_start(
                    xt[b][:], x_d[:, 2 * b * XROW:(2 * b + 2) * XROW])

            def load_y(k):     # 16-row chunks (last: 8 rows)
                r0, r1 = 16 * k, min(16 * k + 16, YH)
                nc.sync.dma_start(y_sb[:, r0 * YW:r1 * YW],
                                  y_d[:, r0 * YW:r1 * YW])

            # prologue: row-0 deps first (the first 16 y rows split across
            # both DGE rings; the first matmuls need only a few x blocks),
            # then the next chunks stream in behind
            nc.sync.dma_start(xt[0][:, 0:512], x_d[:, 0:512])
            nc.scalar.dma_start(y_sb[:, 0:8 * YW], y_d[:, 0:8 * YW])
            nc.sync.dma_start(y_sb[:, 8 * YW:16 * YW],
                              y_d[:, 8 * YW:16 * YW])
            nc.scalar.dma_start(y_sb[:, 16 * YW:32 * YW],
                                y_d[:, 16 * YW:32 * YW])
            nc.sync.dma_start(xt[0][:, 512:], x_d[:, 512:2 * XROW])
            nc.scalar.dma_start(xt[1][:], x_d[:, 2 * XROW:4 * XROW])
            load_y(2)

            def write_row(bh, bandw, glo, ghi):
                # both halves on the sync ring: upper targets the even
                # SDMA engines, lower the odd — they drain concurrently;
                # interleaving makes the slab runs 1152B
                nc.sync.dma_start(
                    out_d[bh, 0:64, glo * 576:ghi * 576],
                    bandw[0:64, glo:ghi, 0:576])
                nc.sync.dma_start(
                    out_d[bh, 64:128, glo * 576:ghi * 576],
                    bandw[64:128, glo:ghi, 192:768])

            prev = None
            for bh in range(NBH):
                b2, r = bh // 2, bh % 2
                # future rows' loads first: no deps, so the in-order DGE
                # queue never stalls on them
                if r == 0:
                    if b2 + 2 < NB2:
                        load_x(b2 + 2)
                    if b2 + 3 < NYC:
                        load_y(b2 + 3)
                band = band_pool.tile([128, NCTX * NBW], bf16)
                bandi = band[:].rearrange("p (j c t) -> p j t c",
                                          t=G, c=NCTX)
                bandw = band[:].rearrange("p (j f) -> p j f", f=NCTX * G)
                for bw in range(NBW):
                    j, t = bw // G, bw % G
                    lhsT = xt[b2][:, r * XROW + bw * 128:
                                  r * XROW + (bw + 1) * 128]
                    rhs = y3[:, PH * bh:PH * bh + CH, PW * bw:PW * bw + CW]
                    ps = psum_pool.tile([128, NCTX], f32)
                    nc.tensor.matmul(ps[:], lhsT, rhs, start=True, stop=True)
                    dst = bandi[:, j, t]
                    # 5:7 ACT:DVE split — ACT pays a higher stride-2 cost
                    if bw in (0, 2, 5, 7, 10):
                        nc.scalar.activation(
                            dst, ps[:], mybir.ActivationFunctionType.Copy)
                    else:
                        nc.vector.tensor_copy(dst, ps[:])
                    # previous row's slabs early in this row: their evac
                    # deps are long satisfied, no queue blocking
                    if bw == 2 and prev is not None:
                        write_row(bh - 1, prev, 0, NG)
                    if bh == NBH - 1 and bw == 8:
                        # last row: progressive pieces so only the final
                        # group drains in the tail
                        write_row(bh, bandw, 0, 3)
                    if bh == NBH - 1 and bw == 11:
                        write_row(bh, bandw, 3, 5)
                prev = bandw
            write_row(NBH - 1, prev, 5, NG)

    nc.compile()
    return nc


def _get_nc():
    global _nc_cache
    if _nc_cache is None:
        _nc_cache = _build_nc()
    return _nc_cache


def shard_inputs(x, y):
    import ml_dtypes
    xs_all = np.asarray(x, dtype=np.float32) * np.float32(1.0 / C)
    xb = xs_all.astype(ml_dtypes.bfloat16)
    yp = np.pad(np.asarray(y).astype(np.float32),
                ((0, 0), (0, 0), (PAD, PAD), (PAD, PAD))
                ).astype(ml_dtypes.bfloat16)
    in_maps = []
    for b in range(B):
        for hh in range(2):
            xs = xb[b, :, hh * HSH:(hh + 1) * HSH, :]     # [c, 96, 192]
            # pre-block: [c, bh, hl, bw, wl] -> [c, (bh bw), (hl wl)]
            xs = xs.reshape(C, NBH, PH, NBW, PW).transpose(0, 1, 3, 2, 4)
            xs = np.ascontiguousarray(xs.reshape(C, NBLK * 128))
            ys = yp[b, :, hh * HSH:hh * HSH + YH, :]      # [c, 104, 200]
            ys = np.ascontiguousarray(ys.reshape(C, YH * YW))
            in_maps.append({"x": xs, "y": ys})
    return in_maps


def _gather_cols():
    # col index into a [128, ROWW] output row for pixel m=(hl,wl) of
    # block bw, offset k=(di,dj): the slab holds ctx rows 0..11 for hl<4
    # and rows 4..15 for hl>=4, interleaved across each pair of blocks:
    #   (bw//G)*288*G + ((hl+di - 4*(hl>=4))*CW + (wl+dj))*G + bw%G
    m = np.arange(128)
    hl, wl = m // PW, m % PW
    di, dj = np.arange(K * K) // K, np.arange(K * K) % K
    base = (hl - 4 * (hl >= 4)) * CW + wl                  # [128]
    q = di * CW + dj                                       # [81]
    bw = np.arange(NBW)
    return ((bw[None, :, None] // G) * 288 * G
            + (base[:, None, None] + q[None, None, :]) * G
            + bw[None, :, None] % G)                       # [128, 12, 81]


_COLS = _gather_cols().reshape(1, 128, NBW * K2)


def unshard_output(results):
    out = np.empty((B, K2, H, W), np.float32)
    for core, r in enumerate(results):
        arr = np.asarray(r["out"])                    # [12, 128, 3456] bf16
        sel = np.take_along_axis(arr, _COLS, axis=2)  # [12, 128, 12*81]
        b, hh = divmod(core, 2)
        o = sel.reshape(NBH, PH, PW, NBW, K2)         # [bh, hl, wl, bw, k]
        o = o.transpose(4, 0, 1, 3, 2).reshape(K2, HSH, W).astype(np.float32)
        out[b, :, hh * HSH:(hh + 1) * HSH, :] = o
    return out


def kernel(x, y, kernel_size, stride, _trace=False):
    assert int(kernel_size) == K and int(stride) == 1
    from concourse.bass_utils import run_bass_kernel_spmd
    nc = _get_nc()
    in_maps = shard_inputs(x, y)
    try:
        res = run_bass_kernel_spmd(nc, in_maps, list(range(NCORES)),
                                   trace=_trace)
    except Exception:
        if not _trace:
            raise
        res = run_bass_kernel_spmd(nc, in_maps, list(range(NCORES)))
    out = unshard_output(res.results)
    if _trace:
        return out, res
    return out
